# revision 9
# baseline (speedup 1.0000x reference)
"""Trainium2 Bass kernel for the AttnEncoder LSTM problem.

Reference computation (per timestep t, PyTorch LSTM cell gate order i,f,g,o):
    z1 = relu([h, c] @ W1.T + b1)          # [B, 512]
    z2 = relu(v_t @ W2.T + b2)             # [B, 512]  (recurrence-independent)
    x  = relu([z1, z2] @ W3.T + b3)        # [B, 512]
    gates = x @ Wih.T + bih + h @ Whh.T + bhh
    c' = sig(f)*c + sig(i)*tanh(g);  h' = sig(o)*tanh(c')
Output: h stacked over t -> [B, T, 512].

Strategy: 8-way data parallel over batch (B=1024 -> 128 rows/core, exactly one
SBUF partition tile). Everything on-device is kept feature-major ([feat, batch])
so activations feed the next matmul as the moving operand with no transposes.
Matmul inputs are bf16 (1 cyc/row on PE vs 4 for fp32); all elementwise state
math is fp32. z2 for all timesteps is precomputed into a DRAM scratch first.
"""

import numpy as np
import ml_dtypes

import concourse.bass as bass
import concourse.mybir as mybir
import concourse.tile as tile
from concourse import bacc
from concourse.bass_utils import run_bass_kernel_spmd

F32 = mybir.dt.float32
BF16 = mybir.dt.bfloat16
AF = mybir.ActivationFunctionType
ts = bass.ts

B, T, DP = 1024, 128, 10
H = 512
NCORES = 8
BL = B // NCORES  # 128 batch rows per core

_CACHE = {}
LAST_RESULTS = None


def build(t_steps=T, do_compile=True):
    nc = bacc.Bacc("TRN2", num_devices=NCORES)

    # Pre-transposed weight chunk layouts (built on host):
    #   w1t[p, (k*4+m)*128+q] = W1[128m+q, 128k+p]      k: [h;c] chunks, m: out chunks
    #   w3t[p, (k*4+m)*128+q] = W3[128m+q, 128k+p]      k: [z1;z2] chunks
    #   wgt[p, (k*16+m)*128+q] = [Wih|Whh][128m+q, 128k+p]
    w1t = nc.dram_tensor("w1t", [128, 32 * 128], BF16, kind="ExternalInput")
    w3t = nc.dram_tensor("w3t", [128, 32 * 128], BF16, kind="ExternalInput")
    wgt = nc.dram_tensor("wgt", [128, 128 * 128], BF16, kind="ExternalInput")
    w2t = nc.dram_tensor("w2t", [DP, 512], BF16, kind="ExternalInput")
    svt = nc.dram_tensor("svt", [DP, T * BL], BF16, kind="ExternalInput")
    b1t = nc.dram_tensor("b1t", [128, 4], F32, kind="ExternalInput")
    b3t = nc.dram_tensor("b3t", [128, 4], F32, kind="ExternalInput")
    bgt = nc.dram_tensor("bgt", [128, 16], F32, kind="ExternalInput")
    b2t = nc.dram_tensor("b2t", [128, 4], F32, kind="ExternalInput")
    # out[t, p, 128m+b] = h_t[feature 128m+p, batch b]
    out = nc.dram_tensor("out", [T, 128, 512], F32, kind="ExternalOutput")
    # z2 scratch: z2d[t, m, p, b] = z2_t[feature 128m+p, batch b] (bf16)
    z2d = nc.dram_tensor("z2d", [T, 4, 128, BL], BF16, kind="Internal")

    with tile.TileContext(nc) as tc:
        with (
            tc.tile_pool(name="weights", bufs=1) as wpool,
            tc.tile_pool(name="state", bufs=2) as spool,
            tc.tile_pool(name="work", bufs=2) as wkpool,
            tc.tile_pool(name="z2in", bufs=3) as z2pool,
            tc.tile_pool(name="psum", bufs=1, space="PSUM") as pp,
        ):
            w1 = wpool.tile([128, 32 * 128], BF16)
            nc.sync.dma_start(w1[:], w1t[:, :])
            w3 = wpool.tile([128, 32 * 128], BF16)
            nc.sync.dma_start(w3[:], w3t[:, :])
            wg = wpool.tile([128, 128 * 128], BF16)
            nc.sync.dma_start(wg[:], wgt[:, :])
            b1s = wpool.tile([128, 4], F32)
            nc.sync.dma_start(b1s[:], b1t[:, :])
            b3s = wpool.tile([128, 4], F32)
            nc.sync.dma_start(b3s[:], b3t[:, :])
            bgs = wpool.tile([128, 16], F32)
            nc.sync.dma_start(bgs[:], bgt[:, :])
            b2s = wpool.tile([128, 4], F32)
            nc.sync.dma_start(b2s[:], b2t[:, :])

            # ---------------- phase 1: z2 for all timesteps ----------------
            with (
                tc.tile_pool(name="z2phase", bufs=1) as zp,
                tc.tile_pool(name="z2psum", bufs=2, space="PSUM") as zpp,
            ):
                w2 = zp.tile([DP, 512], BF16)
                nc.sync.dma_start(w2[:], w2t[:, :])
                sv = zp.tile([DP, T * BL], BF16)
                nc.sync.dma_start(sv[:], svt[:, :])
                for g in range(T * BL // 512):  # 32 groups of 4 timesteps
                    for m in range(4):
                        ps = zpp.tile([128, 512], F32, tag="zps", bufs=2)
                        nc.tensor.matmul(
                            ps[:], w2[:, ts(m, 128)], sv[:, ts(g, 512)],
                            start=True, stop=True,
                        )
                        zs = zp.tile([128, 512], BF16, tag="zs", bufs=3)
                        nc.scalar.activation(
                            zs[:], ps[:], AF.Relu, bias=b2s[:, m : m + 1]
                        )
                        nc.sync.dma_start(
                            z2d[4 * g : 4 * g + 4, m].rearrange("t p b -> p t b"),
                            zs[:].rearrange("p (t b) -> p t b", t=4),
                        )

            # ---------------- phase 2: recurrence over T ----------------
            h_bf = spool.tile([128, 512], BF16, tag="hbf", bufs=2)
            nc.vector.memset(h_bf[:], 0.0)
            c_bf = spool.tile([128, 512], BF16, tag="cbf", bufs=2)
            nc.vector.memset(c_bf[:], 0.0)
            c32 = spool.tile([128, 512], F32, tag="c32", bufs=2)
            nc.vector.memset(c32[:], 0.0)

            funcs = [AF.Sigmoid, AF.Sigmoid, AF.Tanh, AF.Sigmoid]

            for t in range(t_steps):
                z2t = z2pool.tile([128, 512], BF16, tag="z2t", bufs=3)
                nc.sync.dma_start(
                    z2t[:].rearrange("p (m b) -> p m b", m=4),
                    z2d[t].rearrange("m p b -> p m b"),
                )

                # One PSUM accumulation group per bank per step: start=True on
                # the bank's first matmul zeroes the whole 2KB bank; stop=True
                # on the bank's last matmul closes the group.

                # z1 = relu(W1 @ [h; c] + b1), feature-major
                z1ps = pp.tile([128, 512], F32, tag="z1ps", bufs=1)
                for m in range(4):
                    for k in range(8):
                        rhs = h_bf[:, ts(k, 128)] if k < 4 else c_bf[:, ts(k - 4, 128)]
                        nc.tensor.matmul(
                            z1ps[:, ts(m, 128)], w1[:, ts(k * 4 + m, 128)], rhs,
                            start=(m == 0 and k == 0), stop=(m == 3 and k == 7),
                        )

                # gates pass 1: Whh @ h contributions (independent of z1/x) —
                # keeps PE busy while z1/x activations run.
                gps = [
                    pp.tile([128, 512], F32, tag=f"g{i}ps", bufs=1, name=f"g{i}ps")
                    for i in range(4)
                ]
                for gi in range(4):
                    for j in range(4):
                        mm = gi * 4 + j
                        for k in range(4, 8):
                            nc.tensor.matmul(
                                gps[gi][:, ts(j, 128)],
                                wg[:, ts(k * 16 + mm, 128)],
                                h_bf[:, ts(k - 4, 128)],
                                start=(j == 0 and k == 4), stop=False,
                            )

                z1bf = wkpool.tile([128, 512], BF16, tag="z1bf", bufs=2)
                for m in range(4):
                    nc.scalar.activation(
                        z1bf[:, ts(m, 128)], z1ps[:, ts(m, 128)], AF.Relu,
                        bias=b1s[:, m : m + 1],
                    )

                # x = relu(W3 @ [z1; z2] + b3) — all z2 contributions first so
                # the PE has work while the z1 relus run.
                xps = pp.tile([128, 512], F32, tag="xps", bufs=1)
                for m in range(4):
                    for kz in range(4):
                        k = 4 + kz  # z2 chunk
                        nc.tensor.matmul(
                            xps[:, ts(m, 128)], w3[:, ts(k * 4 + m, 128)],
                            z2t[:, ts(kz, 128)],
                            start=(m == 0 and kz == 0), stop=False,
                        )
                for m in range(4):
                    for k in range(4):  # z1 chunks
                        nc.tensor.matmul(
                            xps[:, ts(m, 128)], w3[:, ts(k * 4 + m, 128)],
                            z1bf[:, ts(k, 128)],
                            start=False, stop=(m == 3 and k == 3),
                        )
                xbf = wkpool.tile([128, 512], BF16, tag="xbf", bufs=2)
                for m in range(4):
                    nc.scalar.activation(
                        xbf[:, ts(m, 128)], xps[:, ts(m, 128)], AF.Relu,
                        bias=b3s[:, m : m + 1],
                    )

                # gates pass 2: Wih @ x contributions
                for gi in range(4):
                    for j in range(4):
                        mm = gi * 4 + j
                        for k in range(4):
                            nc.tensor.matmul(
                                gps[gi][:, ts(j, 128)],
                                wg[:, ts(k * 16 + mm, 128)],
                                xbf[:, ts(k, 128)],
                                start=False, stop=(j == 3 and k == 3),
                            )

                gsb = [
                    wkpool.tile([128, 512], F32, tag=f"g{i}sb", bufs=2, name=f"g{i}sb")
                    for i in range(4)
                ]
                for gi in range(4):
                    for j in range(4):
                        mm = gi * 4 + j
                        nc.scalar.activation(
                            gsb[gi][:, ts(j, 128)], gps[gi][:, ts(j, 128)],
                            funcs[gi], bias=bgs[:, mm : mm + 1],
                        )
                i_s, f_s, g_s, o_s = gsb

                t1 = wkpool.tile([128, 512], F32, tag="t1", bufs=2)
                nc.vector.tensor_mul(t1[:], i_s[:], g_s[:])
                t2 = wkpool.tile([128, 512], F32, tag="t2", bufs=2)
                nc.vector.tensor_mul(t2[:], f_s[:], c32[:])
                c32 = spool.tile([128, 512], F32, tag="c32", bufs=2)
                nc.vector.tensor_add(c32[:], t1[:], t2[:])
                th = wkpool.tile([128, 512], F32, tag="th", bufs=2)
                nc.scalar.activation(th[:], c32[:], AF.Tanh)
                h32 = wkpool.tile([128, 512], F32, tag="h32", bufs=2)
                nc.vector.tensor_mul(h32[:], o_s[:], th[:])
                h_bf = spool.tile([128, 512], BF16, tag="hbf", bufs=2)
                nc.vector.tensor_copy(h_bf[:], h32[:])
                c_bf = spool.tile([128, 512], BF16, tag="cbf", bufs=2)
                nc.vector.tensor_copy(c_bf[:], c32[:])

                nc.sync.dma_start(out[t], h32[:])

    if do_compile:
        nc.compile()
    return nc


def _get_nc():
    if "nc" not in _CACHE:
        _CACHE["nc"] = build()
    return _CACHE["nc"]


def kernel(stockvec, W1, b1, W2, b2, W3, b3, Wih, Whh, bih, bhh):
    global LAST_RESULTS
    bf = ml_dtypes.bfloat16
    f32 = np.float32
    stockvec = np.asarray(stockvec, f32)
    W1, b1, W2, b2, W3, b3 = (np.asarray(a, f32) for a in (W1, b1, W2, b2, W3, b3))
    Wih, Whh, bih, bhh = (np.asarray(a, f32) for a in (Wih, Whh, bih, bhh))

    w1t_np = np.ascontiguousarray(
        W1.reshape(4, 128, 8, 128).transpose(3, 2, 0, 1)
    ).reshape(128, 4096).astype(bf)
    w3t_np = np.ascontiguousarray(
        W3.reshape(4, 128, 8, 128).transpose(3, 2, 0, 1)
    ).reshape(128, 4096).astype(bf)
    wcat = np.concatenate([Wih, Whh], axis=1)  # [2048, 1024]
    wgt_np = np.ascontiguousarray(
        wcat.reshape(16, 128, 8, 128).transpose(3, 2, 0, 1)
    ).reshape(128, 16384).astype(bf)
    w2t_np = np.ascontiguousarray(W2.T).astype(bf)  # [10, 512]
    b1t_np = np.ascontiguousarray(b1.reshape(4, 128).T)
    b3t_np = np.ascontiguousarray(b3.reshape(4, 128).T)
    bgt_np = np.ascontiguousarray((bih + bhh).reshape(16, 128).T)
    b2t_np = np.ascontiguousarray(b2.reshape(4, 128).T)

    in_maps = []
    for ci in range(NCORES):
        shard = stockvec[ci * BL : (ci + 1) * BL]  # [BL, T, 10]
        svt_np = np.ascontiguousarray(
            shard.transpose(2, 1, 0).reshape(DP, T * BL)
        ).astype(bf)
        in_maps.append(
            dict(
                w1t=w1t_np, w3t=w3t_np, wgt=wgt_np, w2t=w2t_np, svt=svt_np,
                b1t=b1t_np, b3t=b3t_np, bgt=bgt_np, b2t=b2t_np,
            )
        )

    nc = _get_nc()
    res = run_bass_kernel_spmd(nc, in_maps, core_ids=list(range(NCORES)))
    LAST_RESULTS = res

    outs = []
    for ci in range(NCORES):
        o = res.results[ci]["out"]  # [T, 128, 512]
        o = o.reshape(T, 128, 4, 128).transpose(3, 0, 2, 1).reshape(BL, T, 512)
        outs.append(o)
    return np.ascontiguousarray(np.concatenate(outs, axis=0)).astype(np.float32)
